# revision 1
# baseline (speedup 1.0000x reference)
"""IoU loss kernel for Trainium2, data-parallel over the batch dim on 8 cores.

Math (per reference):
    probs = softmax(inputs, axis=1)                       # (8, 13, 800, 800)
    intersection = sum_pix probs[b, t, h, w]
    total = probs.sum() + Npix                            # probs.sum() == Npix (+fp noise)
    out = 1 - (intersection + smooth) / (total - intersection + smooth)

Device kernel (per core, one batch item), raw Bass with manual semaphores:
    x: (13, 128, 5000) f32 logits, t: (128, 5000) u8 targets.
    Per chunk of N free-elems (double buffered):
      E = exp(X)                 (ACT, one pass over all 13 classes, bf16 out)
      D = sum_c E_c              (DVE takes E0..E2, GPSIMD takes E3..E12)
      e_sel = E[t]               (bit-serial mux tree: 4 u8 bitmasks + 12 copy_predicated)
      acc[:, j] = sum_free(e_sel / D)   (fused tensor_tensor_reduce with divide)
    Host sums the 8x128xNCHUNK partials and forms the scalar.
"""

import numpy as np

_BS, _C, _H, _W = 8, 13, 800, 800
_P = 128
_FREE = (_H * _W) // _P  # 5000
_N = 500                 # chunk free size
_NCHUNK = _FREE // _N    # 10
_NBUF = 3
_NCORES = 8
_NPIX = _BS * _H * _W    # 5120000

# class indices summed by DVE vs GPSIMD for the denominator
_DVE_CLASSES = (0, 1, 2, 3)
_POOL_CLASSES = tuple(range(4, 13))

# mux-tree pairs per bit level: cp(E_a <- E_b where bit_k(t) != 0)
_TREE = [
    (0, [(0, 1), (2, 3), (4, 5), (6, 7), (8, 9), (10, 11)]),
    (1, [(0, 2), (4, 6), (8, 10)]),
    (2, [(0, 4), (8, 12)]),
    (3, [(0, 8)]),
]

_cached = {}


def _build_program():
    from contextlib import ExitStack

    import concourse.bass as bass
    import concourse.mybir as mybir

    f32 = mybir.dt.float32
    bf16 = mybir.dt.bfloat16
    u8 = mybir.dt.uint8
    Alu = mybir.AluOpType
    Act = mybir.ActivationFunctionType

    nc = bass.Bass(trn_type="TRN2")
    x = nc.declare_dram_parameter("x", [_C, _P, _FREE], f32, isOutput=False)
    t = nc.declare_dram_parameter("t", [_P, _FREE], u8, isOutput=False)
    part = nc.declare_dram_parameter("part", [_P, _NCHUNK], f32, isOutput=True)

    ctx = ExitStack()
    with ctx:
        T = ctx.enter_context(nc.sbuf_tensor("T", [_P, _FREE], u8))
        acc = ctx.enter_context(nc.sbuf_tensor("acc", [_P, _NCHUNK], f32))
        dummy = ctx.enter_context(nc.sbuf_tensor("ttr_dummy", [_P, 1], f32))
        X = [ctx.enter_context(nc.sbuf_tensor(f"X{i}", [_P, _C, _N], f32))
             for i in range(_NBUF)]
        E = [ctx.enter_context(nc.sbuf_tensor(f"E{i}", [_P, _C, _N], bf16))
             for i in range(_NBUF)]
        D1 = [ctx.enter_context(nc.sbuf_tensor(f"D1_{i}", [_P, _N], bf16))
              for i in range(_NBUF)]
        D1b = [ctx.enter_context(nc.sbuf_tensor(f"D1b_{i}", [_P, _N], bf16))
               for i in range(_NBUF)]
        D2 = [ctx.enter_context(nc.sbuf_tensor(f"D2_{i}", [_P, _N], bf16))
              for i in range(_NBUF)]
        S = [[ctx.enter_context(nc.sbuf_tensor(f"S{i}_{k}", [_P, _N], bf16))
              for k in range(4)] for i in range(_NBUF)]
        Dm = [ctx.enter_context(nc.sbuf_tensor(f"Dm_{i}", [_P, _N], bf16))
              for i in range(_NBUF)]
        R = [ctx.enter_context(nc.sbuf_tensor(f"R_{i}", [_P, _N], f32))
             for i in range(_NBUF)]
        M = [[ctx.enter_context(nc.sbuf_tensor(f"M{i}_{k}", [_P, _N], u8))
              for k in range(4)] for i in range(_NBUF)]

        block = ctx.enter_context(nc.Block())
        dma_t = ctx.enter_context(nc.semaphore("dma_t"))
        dma_x = [ctx.enter_context(nc.semaphore(f"dma_x{i}"))
                 for i in range(_NBUF)]
        dma_out = ctx.enter_context(nc.semaphore("dma_out"))
        s_exp = ctx.enter_context(nc.semaphore("s_exp"))
        s_d2 = ctx.enter_context(nc.semaphore("s_d2"))
        s_ttr = ctx.enter_context(nc.semaphore("s_ttr"))

        @block.sync
        def _(sync):
            sync.dma_start(out=T[:, :], in_=t[:, :]).then_inc(dma_t, 16)
            for j in range(_NCHUNK):
                b = j % _NBUF
                rnd = j // _NBUF
                if j >= _NBUF:
                    # X[b] is reread by exp of chunk j-NBUF; exp-done
                    # implies X free and implies the slot's previous DMA
                    # completed (so the wait below is satisfied already —
                    # it exists to order updates of the per-slot sem).
                    sync.wait_ge(s_exp, j - _NBUF + 1)
                    sync.wait_ge(dma_x[b], 16 * rnd)
                sync.dma_start(
                    out=X[b][:, :, :],
                    in_=x[:, :, j * _N:(j + 1) * _N].transpose([1, 0, 2]),
                ).then_inc(dma_x[b], 16)
            sync.wait_ge(s_ttr, _NCHUNK)
            sync.dma_start(out=part[:, :], in_=acc[:, :]).then_inc(dma_out, 16)
            sync.wait_ge(dma_out, 16)

        @block.scalar
        def _(scalar):
            for j in range(_NCHUNK):
                b = j % _NBUF
                scalar.wait_ge(dma_x[b], 16 * (j // _NBUF + 1))
                if j >= _NBUF:
                    # E[b] readers from chunk j-NBUF: DVE (ttr last) + POOL
                    scalar.wait_ge(s_ttr, j - _NBUF + 1)
                    scalar.wait_ge(s_d2, j - _NBUF + 1)
                scalar.activation(
                    out=E[b][:, :, :], in_=X[b][:, :, :], func=Act.Exp
                ).then_inc(s_exp, 1)

        @block.gpsimd
        def _(gpsimd):
            for j in range(_NCHUNK):
                b = j % _NBUF
                gpsimd.wait_ge(s_exp, j + 1)
                if j >= _NBUF:
                    # D2[b]/S[b] are read by DVE merge of chunk j-NBUF
                    gpsimd.wait_ge(s_ttr, j - _NBUF + 1)
                # classes 4..12: pairwise tree to minimize drains
                Eb = E[b]
                Sb = S[b]
                gpsimd.tensor_tensor(out=Sb[0][:, :], in0=Eb[:, 4, :],
                                     in1=Eb[:, 5, :], op=Alu.add)
                gpsimd.tensor_tensor(out=Sb[1][:, :], in0=Eb[:, 6, :],
                                     in1=Eb[:, 7, :], op=Alu.add)
                gpsimd.tensor_tensor(out=Sb[2][:, :], in0=Eb[:, 8, :],
                                     in1=Eb[:, 9, :], op=Alu.add)
                gpsimd.tensor_tensor(out=Sb[3][:, :], in0=Eb[:, 10, :],
                                     in1=Eb[:, 11, :], op=Alu.add)
                gpsimd.drain()
                gpsimd.tensor_tensor(out=Sb[0][:, :], in0=Sb[0][:, :],
                                     in1=Sb[1][:, :], op=Alu.add)
                gpsimd.tensor_tensor(out=Sb[2][:, :], in0=Sb[2][:, :],
                                     in1=Sb[3][:, :], op=Alu.add)
                gpsimd.drain()
                gpsimd.tensor_tensor(out=Sb[0][:, :], in0=Sb[0][:, :],
                                     in1=Sb[2][:, :], op=Alu.add)
                gpsimd.drain()
                gpsimd.tensor_tensor(
                    out=D2[b][:, :], in0=Sb[0][:, :], in1=Eb[:, 12, :],
                    op=Alu.add
                ).then_inc(s_d2, 1)

        @block.vector
        def _(vector):
            vector.wait_ge(dma_t, 16)
            for j in range(_NCHUNK):
                b = j % _NBUF
                # masks only need T — no wait
                Tc = T[:, j * _N:(j + 1) * _N]
                for k in range(4):
                    vector.tensor_scalar(M[b][k][:, :], Tc, 1 << k, None,
                                         Alu.bitwise_and)
                vector.wait_ge(s_exp, j + 1)
                # classes 0..3 pairwise, then merge with POOL's D2
                vector.tensor_tensor(
                    out=D1[b][:, :], in0=E[b][:, 0, :], in1=E[b][:, 1, :],
                    op=Alu.add)
                vector.tensor_tensor(
                    out=D1b[b][:, :], in0=E[b][:, 2, :], in1=E[b][:, 3, :],
                    op=Alu.add)
                vector.drain()
                vector.tensor_tensor(
                    out=D1[b][:, :], in0=D1[b][:, :], in1=D1b[b][:, :],
                    op=Alu.add)
                vector.wait_ge(s_d2, j + 1)
                vector.drain()
                vector.tensor_tensor(out=Dm[b][:, :], in0=D1[b][:, :],
                                     in1=D2[b][:, :], op=Alu.add)
                vector.drain()
                vector.reciprocal(R[b][:, :], Dm[b][:, :])
                for k, pairs in _TREE:
                    for a, c in pairs:
                        vector.copy_predicated(
                            E[b][:, a, :], M[b][k][:, :], E[b][:, c, :])
                    vector.drain()
                vector.scalar_tensor_tensor(
                    out=dummy[:, :].broadcast_to((_P, _N)),
                    in0=E[b][:, 0, :],
                    scalar=1.0,
                    in1=R[b][:, :],
                    op0=Alu.bypass,
                    op1=Alu.mult,
                    accum_out=acc[:, j:j + 1],
                ).then_inc(s_ttr, 1)

    return nc


def _get_program():
    if "nc" not in _cached:
        _cached["nc"] = _build_program()
    return _cached["nc"]


def _make_in_maps(inputs, targets):
    in_maps = []
    for b in range(_NCORES):
        xb = np.ascontiguousarray(inputs[b]).reshape(_C, _P, _FREE)
        tb = np.ascontiguousarray(targets[b]).astype(np.uint8).reshape(_P, _FREE)
        in_maps.append({"x": xb, "t": tb})
    return in_maps


def _finalize(parts, smooth):
    inter = 0.0
    for p in parts:
        inter += float(np.sum(np.asarray(p).astype(np.float64)))
    s = float(smooth)
    total = 2.0 * float(_NPIX)
    union = total - inter
    out = 1.0 - (inter + s) / (union + s)
    return np.asarray(np.float32(out))


def kernel(inputs, targets, smooth):
    from concourse.bass_utils import run_bass_kernel_spmd

    nc = _get_program()
    in_maps = _make_in_maps(np.asarray(inputs), np.asarray(targets))
    res = run_bass_kernel_spmd(nc, in_maps, list(range(_NCORES)))
    return _finalize([res.results[b]["part"] for b in range(_NCORES)], smooth)

